# revision 4
# baseline (speedup 1.0000x reference)
"""CTC greedy decode (merge repeats, drop blank) on 8 Trainium2 cores.

Input : y_pred [256, 2048, 80] f32
Output: [256, 2048] int32, left-aligned decoded ids padded with -1.

Sharding: pure data-parallel, 32 sequences per core.

Per-core device pipeline (B=32 seqs, N=65536 flat (b,t) rows):
  1. Stream y in 16 chunks of [128, 32*80]; batched 3D reduce_max over the
     class axis -> m[128, 512] (per-row max).
  2. Per 128-row tile: scalar_tensor_tensor (y >= m) * w, w[c] = 80-c, with
     sum-accumulate -> r[128, 512] where r = 80 - argmax (exact when the row
     max is unique; tied rows are repaired on host via the m/r side outputs).
  3. PE-transpose r into S[t][block, tau] (time-major): partition n = block of
     128 consecutive tau, seq = (128*t + n) // 16. Compute keep flags; then
     compact each 8-element tau-group with the Max8 unit using a composite
     encoding keep * ((7 - tau%8)*256 + ids + 1): descending sort = stable
     compaction with zero tails. Group lengths -> prefix scan -> run offsets
     (PE matmul for the cross-partition block carry).
  4. One indirect-DMA per group column scatters 8-element runs (one run per
     partition) at their global offsets with accumulate-add onto the
     zero-initialized f32 output; zero tails make overlaps harmless. The host
     rounds, subtracts 1 (empty slots 0 -> -1).
"""

import numpy as np

B, T, C = 256, 2048, 80
NCORES = 8
B_CORE = B // NCORES            # 32 seqs per core
N = B_CORE * T                  # 65536 flat rows per core
TILES = N // 128                # 512
CHUNK_TILES = 32                # tiles per chunk
NCHUNK = TILES // CHUNK_TILES   # 16
OUT_PAD = N + 8

_cache = {}


def _build_nc():
    import concourse.bacc as bacc
    import concourse.mybir as mybir
    from concourse import bass
    from concourse.tile import TileContext

    f32 = mybir.dt.float32
    i32 = mybir.dt.int32
    Alu = mybir.AluOpType
    Act = mybir.ActivationFunctionType

    nc = bacc.Bacc("TRN2")
    y = nc.dram_tensor("y", [N, C], f32, kind="ExternalInput")
    wvec = nc.dram_tensor("wvec", [128, C], f32, kind="ExternalInput")
    t16 = nc.dram_tensor("t16", [128, 128], f32, kind="ExternalInput")
    sub16 = nc.dram_tensor("sub16", [128, 128], f32, kind="ExternalInput")
    seqb = nc.dram_tensor("seqb", [128, 4], f32, kind="ExternalInput")
    ident = nc.dram_tensor("ident", [128, 128], f32, kind="ExternalInput")
    p8 = nc.dram_tensor("p8", [128, 128], f32, kind="ExternalInput")
    out = nc.dram_tensor("out", [1, OUT_PAD], f32, kind="ExternalOutput")
    m_out = nc.dram_tensor("m_out", [128, TILES], f32, kind="ExternalOutput")
    r_out = nc.dram_tensor("r_out", [128, TILES], f32, kind="ExternalOutput")

    # DRAM view: chunk c, partition p, tile-in-chunk j, class k
    y_re = y[:].rearrange("(c j p) k -> c p j k", c=NCHUNK, j=CHUNK_TILES, p=128)

    with TileContext(nc) as tc:
        with (
            tc.tile_pool(name="ypool", bufs=3) as ypool,
            tc.tile_pool(name="scratch", bufs=4) as spool,
            tc.tile_pool(name="persist", bufs=1) as ppool,
            tc.tile_pool(name="small", bufs=2) as smpool,
            tc.tile_pool(name="psum", bufs=2, space="PSUM") as psum,
        ):
            # constants
            w_sb = ppool.tile([128, C], f32, tag="w")
            nc.sync.dma_start(out=w_sb[:], in_=wvec[:])
            t16_sb = ppool.tile([128, 128], f32, tag="t16")
            nc.sync.dma_start(out=t16_sb[:], in_=t16[:])
            sub16_sb = ppool.tile([128, 128], f32, tag="sub16")
            nc.sync.dma_start(out=sub16_sb[:], in_=sub16[:])
            seqb_sb = ppool.tile([128, 4], f32, tag="seqb")
            nc.sync.dma_start(out=seqb_sb[:], in_=seqb[:])
            id_sb = ppool.tile([128, 128], f32, tag="ident")
            nc.sync.dma_start(out=id_sb[:], in_=ident[:])
            p8_sb = ppool.tile([128, 128], f32, tag="p8")
            nc.sync.dma_start(out=p8_sb[:], in_=p8[:])

            m_sb = ppool.tile([128, TILES], f32, tag="m")
            r_sb = ppool.tile([128, TILES], f32, tag="r")

            # ---- stage 1+2: max and argmax-sum per 128-row tile ----
            for ci in range(NCHUNK):
                yt = ypool.tile([128, CHUNK_TILES * C], f32, tag="y")
                nc.sync.dma_start(out=yt[:], in_=y_re[ci])
                y3 = yt[:].rearrange("p (j k) -> p j k", k=C)
                c0 = ci * CHUNK_TILES
                nc.vector.tensor_reduce(
                    out=m_sb[:, c0:c0 + CHUNK_TILES], in_=y3,
                    axis=mybir.AxisListType.X, op=Alu.max,
                )
                for j in range(CHUNK_TILES):
                    sc = spool.tile([128, C], f32, tag="sc")
                    nc.vector.scalar_tensor_tensor(
                        out=sc[:],
                        in0=yt[:, j * C:(j + 1) * C],
                        scalar=m_sb[:, c0 + j:c0 + j + 1],
                        in1=w_sb[:],
                        op0=Alu.is_ge,
                        op1=Alu.mult,
                        accum_out=r_sb[:, c0 + j:c0 + j + 1],
                    )

            nc.sync.dma_start(out=m_out[:], in_=m_sb[:])
            nc.sync.dma_start(out=r_out[:], in_=r_sb[:])

            # ---- stage 3: transpose to time-major, keep, compact, scatter ----
            for t in range(4):
                rT_ps = psum.tile([128, 128], f32, space="PSUM", tag="rT")
                nc.tensor.transpose(
                    out=rT_ps[:], in_=r_sb[:, t * 128:(t + 1) * 128],
                    identity=id_sb[:],
                )
                S = smpool.tile([128, 128], f32, tag="S")
                nc.vector.tensor_copy(S[:], rT_ps[:])

                # prevcol[n] = S[n-1, 127] if n%16 else 0 (seq-start sentinel)
                pc_ps = psum.tile([128, 1], f32, space="PSUM", tag="pc")
                nc.tensor.matmul(
                    out=pc_ps[:], lhsT=sub16_sb[:], rhs=S[:, 127:128],
                    start=True, stop=True,
                )
                pc = smpool.tile([128, 1], f32, tag="pcs")
                nc.vector.tensor_copy(pc[:], pc_ps[:])

                # keep = (r != 1) & (r != prev)
                k1 = smpool.tile([128, 128], f32, tag="k1")
                nc.vector.tensor_scalar(
                    k1[:], S[:], 1.0, None, op0=Alu.not_equal)
                k2 = smpool.tile([128, 128], f32, tag="k2")
                nc.vector.tensor_tensor(
                    out=k2[:, 1:128], in0=S[:, 1:128], in1=S[:, 0:127],
                    op=Alu.not_equal)
                nc.vector.tensor_tensor(
                    out=k2[:, 0:1], in0=S[:, 0:1], in1=pc[:],
                    op=Alu.not_equal)
                keep = smpool.tile([128, 128], f32, tag="keep")
                nc.vector.tensor_tensor(
                    out=keep[:], in0=k1[:], in1=k2[:], op=Alu.mult)

                # values to emit: ids + 1 = 81 - r  (on ScalarE)
                val_f = smpool.tile([128, 128], f32, tag="val_f")
                nc.scalar.activation(
                    out=val_f[:], in_=S[:], func=Act.Copy, bias=81.0,
                    scale=-1.0)

                # composite = keep * ((7 - tau%8)*256 + ids + 1)
                u1 = smpool.tile([128, 128], f32, tag="u1")
                nc.vector.tensor_tensor(
                    out=u1[:], in0=val_f[:], in1=p8_sb[:], op=Alu.add)
                comp = smpool.tile([128, 128], f32, tag="comp")
                nc.vector.tensor_tensor(
                    out=comp[:], in0=u1[:], in1=keep[:], op=Alu.mult)

                # compact each 8-group: descending Max8 sort -> kept in tau
                # order, then zero tail
                cruns = smpool.tile([128, 128], f32, tag="cruns")
                for g in range(16):
                    nc.vector.max(
                        out=cruns[:, g * 8:(g + 1) * 8],
                        in_=comp[:, g * 8:(g + 1) * 8])

                # group lengths and exclusive scan -> within-partition offsets
                ng = smpool.tile([128, 16], f32, tag="ng")
                nc.vector.tensor_reduce(
                    out=ng[:], in_=keep[:].rearrange("p (g e) -> p g e", e=8),
                    axis=mybir.AxisListType.X, op=Alu.add)
                og = smpool.tile([128, 17], f32, tag="og")
                nc.vector.memset(og[:, 0:1], 0.0)
                nc.vector.tensor_tensor_scan(
                    out=og[:, 1:17], data0=ng[:], data1=ng[:], initial=0.0,
                    op0=Alu.add, op1=Alu.bypass)

                # cross-partition carry within each 16-partition seq group
                ca_ps = psum.tile([128, 1], f32, space="PSUM", tag="ca")
                nc.tensor.matmul(
                    out=ca_ps[:], lhsT=t16_sb[:], rhs=og[:, 16:17],
                    start=True, stop=True,
                )
                # c3 = carry + seqbase
                c3 = smpool.tile([128, 1], f32, tag="c3")
                nc.vector.scalar_tensor_tensor(
                    out=c3[:], in0=ca_ps[:], scalar=0.0,
                    in1=seqb_sb[:, t:t + 1], op0=Alu.add, op1=Alu.add)

                # run offsets
                orf = smpool.tile([128, 16], f32, tag="orf")
                nc.vector.tensor_scalar(
                    orf[:], og[:, 0:16], c3[:], None, op0=Alu.add)
                off_i = smpool.tile([128, 16], i32, tag="off_i")
                nc.vector.tensor_copy(off_i[:], orf[:])

                # decode composites back to values (ids+1, or 0 for tail)
                sc1 = smpool.tile([128, 128], f32, tag="sc1")
                nc.vector.tensor_scalar(
                    sc1[:], cruns[:], 1.0 / 256.0, None, op0=Alu.mult)
                qi = smpool.tile([128, 128], i32, tag="qi")
                nc.vector.tensor_copy(qi[:], sc1[:])
                qf = smpool.tile([128, 128], f32, tag="qf")
                nc.vector.tensor_copy(qf[:], qi[:])
                vruns = smpool.tile([128, 128], f32, tag="vruns")
                nc.vector.scalar_tensor_tensor(
                    out=vruns[:], in0=qf[:], scalar=-256.0,
                    in1=cruns[:], op0=Alu.mult, op1=Alu.add)

                for g in range(16):
                    nc.gpsimd.indirect_dma_start(
                        out=out[:],
                        out_offset=bass.IndirectOffsetOnAxis(
                            ap=off_i[:, g:g + 1], axis=1),
                        in_=vruns[:, g * 8:(g + 1) * 8],
                        in_offset=None,
                        compute_op=Alu.add,
                    )

    nc.finalize()
    return nc


def _consts():
    k = np.arange(128)
    w = np.tile((C - np.arange(C, dtype=np.float32)), (128, 1))  # 80-c
    t16 = (((k[:, None] // 16) == (k[None, :] // 16)) &
           (k[:, None] < k[None, :])).astype(np.float32)
    sub16 = ((k[:, None] == (k[None, :] - 1)) &
             ((k[None, :] % 16) != 0)).astype(np.float32)
    seqb = np.empty((128, 4), np.float32)
    for t in range(4):
        seqb[:, t] = ((128 * t + k) // 16) * T
    ident = np.eye(128, dtype=np.float32)
    p8 = np.tile((7 - np.arange(128) % 8).astype(np.float32) * 256.0,
                 (128, 1))
    return {"wvec": w, "t16": t16, "sub16": sub16, "seqb": seqb,
            "ident": ident, "p8": p8}


def _reference_rows(y_rows):
    """Exact numpy replica of the reference decode for [n, T, C] rows."""
    n, t, c = y_rows.shape
    blank = c - 1
    ids = y_rows.argmax(axis=-1).astype(np.int32)
    prev = np.concatenate([np.full((n, 1), -1, np.int32), ids[:, :-1]], axis=1)
    keep = (ids != blank) & (ids != prev)
    pos = np.cumsum(keep, axis=1) - 1
    out = np.full((n, t), -1, np.int32)
    rows, cols = np.nonzero(keep)
    out[rows, pos[rows, cols]] = ids[rows, cols]
    return out


def kernel(y_pred: np.ndarray) -> np.ndarray:
    from concourse.bass_utils import run_bass_kernel_spmd

    if "nc" not in _cache:
        _cache["nc"] = _build_nc()
        _cache["consts"] = _consts()
    nc = _cache["nc"]
    consts = _cache["consts"]

    y_pred = np.ascontiguousarray(y_pred, dtype=np.float32)
    y_cores = y_pred.reshape(NCORES, N, C)
    in_maps = [dict(consts, y=y_cores[i]) for i in range(NCORES)]

    res = run_bass_kernel_spmd(nc, in_maps, core_ids=list(range(NCORES)))

    out_full = np.empty((B, T), np.int32)
    for i in range(NCORES):
        r = res.results[i]
        out_core = np.rint(r["out"].ravel()[:N]).astype(np.int32)
        out_core = out_core.reshape(B_CORE, T) - 1
        # --- host-side verification/repair for tied-max rows ---
        # flat row g lives at (g % 128, g // 128) in the [128, TILES] outputs
        r_flat = np.ascontiguousarray(r["r_out"].T).ravel()
        m_flat = np.ascontiguousarray(r["m_out"].T).ravel()
        ids_dec = np.rint(C - r_flat).astype(np.int64)
        badrange = (ids_dec < 0) | (ids_dec > C - 1)
        idc = np.clip(ids_dec, 0, C - 1)
        y_flat = y_cores[i]
        bad = badrange | (y_flat[np.arange(N), idc] != m_flat)
        if bad.any():
            seqs = np.unique(np.nonzero(bad)[0] // T)
            fixed = _reference_rows(y_flat.reshape(B_CORE, T, C)[seqs])
            out_core[seqs] = fixed
        out_full[i * B_CORE:(i + 1) * B_CORE] = out_core
    return out_full
